# revision 11
# baseline (speedup 1.0000x reference)
"""ViT transformer block (B=64, N=197, D=768, H=12, MLP 3072) on 8 trn2 cores.

Data-parallel over batch (8 images per core). Per core:
  - LayerNorm affine terms folded into the following matmul weights (host).
  - Decoupled rel-pos bias folded into the QK matmul via 30 extra contraction
    dims (one-hot row/col encodings x bias-table slices): scores leave the PE
    with the bias already added.
  - Scores computed transposed (sT[kt, qt]); softmax denominators fall out of
    the AV matmul via a block of 64 ones columns appended to V (AV output rows
    64:128 = broadcast denominators); normalize is one DVE divide per head.
  - q scale folded into Wq; v_bias folded into proj bias (host).
  - bf16 operands into the PE, fp32 accumulation in PSUM.
"""

import numpy as np
import ml_dtypes

import concourse.bass as bass
import concourse.mybir as mybir
import concourse.tile as tile
from concourse import bacc
from concourse.bass_utils import run_bass_kernel_spmd
from concourse.masks import make_identity

F32 = mybir.dt.float32
BF16 = mybir.dt.bfloat16
NPBF16 = ml_dtypes.bfloat16

DIM = 768
HEADS = 12
HD = 64
W0 = 14
W1 = 14
NT = W0 * W1
N = NT + 1  # 197
HID = 4 * DIM  # 3072
B = 64
SCALE = HD ** -0.5
EPS = 1e-6

NCORES = 8
NB = B // NCORES            # 8 images per core
NTOK = NB * N               # 1576
NTILES = 13                 # token tiles of 128
NTOKP = NTILES * 128        # 1664
KEXT = 30                   # extra contraction dims carrying the rel-pos bias
NSL = [512, 512, 512, 128]  # token-column slices of NTOKP
NSL2 = [256] * 6 + [128]    # MLP token-column slices

_nc_cache = {}


def _host_prep(inp):
    """Fold norms/scale/biases; build the rel-pos extension tables."""
    f32 = np.float32
    qkv_w = np.asarray(inp["qkv_w"], f32)
    n1w = np.asarray(inp["norm1_w"], f32)
    n1b = np.asarray(inp["norm1_b"], f32)
    q_bias = np.asarray(inp["q_bias"], f32)
    v_bias = np.asarray(inp["v_bias"], f32)
    proj_w = np.asarray(inp["proj_w"], f32)
    proj_b = np.asarray(inp["proj_b"], f32)
    n2w = np.asarray(inp["norm2_w"], f32)
    n2b = np.asarray(inp["norm2_b"], f32)
    fc1_w = np.asarray(inp["fc1_w"], f32)
    fc1_b = np.asarray(inp["fc1_b"], f32)
    fc2_w = np.asarray(inp["fc2_w"], f32)
    fc2_b = np.asarray(inp["fc2_b"], f32)
    rpb_h = np.asarray(inp["rpb_high"], f32)   # [30, 12]
    rpb_w = np.asarray(inp["rpb_width"], f32)  # [30, 12]

    # qkv with norm1 affine folded; q part pre-scaled
    w_qkv = qkv_w * n1w[None, :]                      # [2304, 768]
    b_qkv = qkv_w @ n1b
    b_qkv[:DIM] += q_bias
    b_qkv[2 * DIM:] += v_bias
    w_qkv[:DIM] *= SCALE
    b_qkv[:DIM] *= SCALE
    wqkv_h = np.ascontiguousarray(
        w_qkv.T.reshape(6, 128, 3 * DIM).transpose(1, 0, 2)).astype(NPBF16)
    qkb_h = np.ascontiguousarray(
        b_qkv[:2 * DIM].reshape(12, 128).T).astype(f32)   # [128, 12]

    # proj; v_bias folded into bias
    pb = proj_b + proj_w @ v_bias                      # [768]
    wproj_h = np.ascontiguousarray(
        proj_w.T.reshape(6, 128, DIM).transpose(1, 0, 2)).astype(NPBF16)

    # fc1 with norm2 folded
    w1 = fc1_w * n2w[None, :]
    b1 = fc1_b + fc1_w @ n2b                           # [3072]
    w1_h = np.ascontiguousarray(
        w1.T.reshape(6, 128, HID).transpose(1, 0, 2)).astype(NPBF16)
    b1_h = np.ascontiguousarray(b1.reshape(24, 128).T).astype(f32)  # [128, 24]

    w2_h = np.ascontiguousarray(
        fc2_w.T.reshape(24, 128, DIM).transpose(1, 0, 2)).astype(NPBF16)
    f2b = fc2_b.astype(f32)
    has_f2b = bool(np.any(f2b != 0.0))

    # --- rel-pos bias factorization ---------------------------------------
    # bias[h,q,k] = rpb_h[high_idx[q,k],h] + rpb_w[width_idx[q,k],h];
    # interior: high_idx = krow-qrow+13. CLS handled by dims 28/29.
    qext = np.zeros((KEXT, N), f32)
    for t in range(N):
        if t == 0:
            qext[28, t] = 1.0
        else:
            p = t - 1
            qext[p // W1, t] = 1.0
            qext[14 + p % W1, t] = 1.0
            qext[29, t] = 1.0
    kext = np.zeros((HEADS, KEXT, N), f32)
    for t in range(N):
        if t == 0:
            kext[:, 28, t] = rpb_h[2 * W0 + 1] + rpb_w[2 * W1 + 1]   # corner
            kext[:, 29, t] = rpb_h[2 * W0] + rpb_w[2 * W1]
        else:
            p = t - 1
            kr, kc = p // W1, p % W1
            for rq in range(W0):
                kext[:, rq, t] = rpb_h[kr - rq + W0 - 1]
            for cq in range(W1):
                kext[:, 14 + cq, t] = rpb_w[kc - cq + W1 - 1]
            kext[:, 28, t] = rpb_h[2 * W0 - 1] + rpb_w[2 * W1 - 1]

    return {
        "wqkv": wqkv_h, "qkb": qkb_h, "wproj": wproj_h,
        "pb": pb.reshape(1, DIM).astype(f32),
        "w1": w1_h, "b1c": b1_h, "w2": w2_h,
        "f2b": f2b.reshape(1, DIM), "has_f2b": has_f2b,
        "qext": qext.astype(NPBF16),
        "kext": np.ascontiguousarray(kext).astype(NPBF16),
    }


def _ln_apply(nc, pool, x_ap, out_ap, eps_col):
    """LayerNorm (no affine) of x_ap [128, 768] f32 -> out_ap bf16."""
    stats = pool.tile([128, 3, 6], F32, tag="ln_stats")
    for sg in range(3):
        nc.vector.bn_stats(stats[:, sg], x_ap[:, sg * 256:(sg + 1) * 256])
    mv = pool.tile([128, 2], F32, tag="ln_mv")
    nc.vector.bn_aggr(mv, stats)
    std = pool.tile([128, 1], F32, tag="ln_std")
    nc.scalar.activation(std, mv[:, 1:2], mybir.ActivationFunctionType.Sqrt,
                         bias=eps_col)
    rstd = pool.tile([128, 1], F32, tag="ln_rstd")
    nc.vector.reciprocal(rstd, std)
    nc.vector.tensor_scalar(out=out_ap, in0=x_ap,
                            scalar1=mv[:, 0:1], scalar2=rstd,
                            op0=mybir.AluOpType.subtract,
                            op1=mybir.AluOpType.mult)


def _build(has_f2b):
    nc = bacc.Bacc("TRN2", target_bir_lowering=False, debug=False,
                   num_devices=NCORES)
    x_d = nc.dram_tensor("x", [NTOK, DIM], F32, kind="ExternalInput")
    wqkv_d = nc.dram_tensor("wqkv", [128, 6, 3 * DIM], BF16, kind="ExternalInput")
    qkb_d = nc.dram_tensor("qkb", [128, 12], F32, kind="ExternalInput")
    wproj_d = nc.dram_tensor("wproj", [128, 6, DIM], BF16, kind="ExternalInput")
    pb_d = nc.dram_tensor("pb", [1, DIM], F32, kind="ExternalInput")
    w1_d = nc.dram_tensor("w1", [128, 6, HID], BF16, kind="ExternalInput")
    b1_d = nc.dram_tensor("b1c", [128, 24], F32, kind="ExternalInput")
    w2_d = nc.dram_tensor("w2", [128, 24, DIM], BF16, kind="ExternalInput")
    if has_f2b:
        f2b_d = nc.dram_tensor("f2b", [1, DIM], F32, kind="ExternalInput")
    qext_d = nc.dram_tensor("qext", [KEXT, N], BF16, kind="ExternalInput")
    kext_d = nc.dram_tensor("kext", [HEADS, KEXT, N], BF16,
                            kind="ExternalInput")
    y_d = nc.dram_tensor("y", [NTOK, DIM], F32, kind="ExternalOutput")

    Act = mybir.ActivationFunctionType
    Alu = mybir.AluOpType

    with tile.TileContext(nc) as tc:
        with (
            tc.tile_pool(name="consts", bufs=1) as cp,
            tc.tile_pool(name="wts", bufs=2) as wp,
            tc.tile_pool(name="small", bufs=4) as sp,
            tc.tile_pool(name="xio", bufs=3) as xp,
            tc.tile_pool(name="big", bufs=1) as bp,
            tc.tile_pool(name="perimg", bufs=2) as ip,
            tc.tile_pool(name="gelu", bufs=1) as gp,
            tc.tile_pool(name="ptile", bufs=4) as pp,
            tc.tile_pool(name="dram", bufs=1, space="DRAM") as dp,
            tc.tile_pool(name="psA", bufs=2, space="PSUM") as psA,
            tc.tile_pool(name="psT", bufs=2, space="PSUM") as psT,
            tc.tile_pool(name="psS", bufs=4, space="PSUM") as psS,
        ):
            # ---- constants -------------------------------------------------
            ident = cp.tile([128, 128], BF16)
            make_identity(nc, ident)
            eps_col = cp.tile([128, 1], F32)
            nc.vector.memset(eps_col, EPS)
            qkb_sb = cp.tile([128, 12], F32)
            nc.sync.dma_start(qkb_sb, qkb_d[:])
            b1_sb = cp.tile([128, 24], F32)
            nc.sync.dma_start(b1_sb, b1_d[:])
            pb_sb = cp.tile([128, DIM], F32)
            nc.sync.dma_start(pb_sb, pb_d[:].to_broadcast((128, DIM)))
            if has_f2b:
                f2b_sb = cp.tile([128, DIM], F32)
                nc.sync.dma_start(f2b_sb, f2b_d[:].to_broadcast((128, DIM)))
            # big weights share one rotating 2-slot tag:
            # wqkv (phases A-C) + wproj (D) -> then w1 + w2 (F)
            wqkv_sb = wp.tile([128, 6, 3 * DIM], BF16, tag="wbig")
            nc.sync.dma_start(wqkv_sb, wqkv_d[:])
            wproj_sb = wp.tile([128, 6, DIM], BF16, tag="wbig")
            nc.sync.dma_start(wproj_sb, wproj_d[:])

            # ---- persistent activations -----------------------------------
            hT = bp.tile([128, 6, NTOKP], BF16, tag="featmaj")
            attn_oT = bp.tile([128, 6, NTOKP], BF16)
            h2T = bp.tile([128, 6, NTOKP], BF16, tag="featmaj")
            # NB: h2T shares featmaj with hT -> its writes wait for hT's last
            # reader (the per-image qkv matmuls), which precede phase E anyway.
            x1_dram = dp.tile([NTOKP, DIM], F32)

            # ---- phase A: LN1 + transpose to hT ---------------------------
            for t in range(NTILES):
                rows = min(128, NTOK - t * 128)
                x_t = xp.tile([128, DIM], F32, tag="x_t")
                if rows < 128:
                    nc.vector.memset(x_t, 0.0)
                nc.sync.dma_start(x_t[0:rows], x_d[t * 128:t * 128 + rows])
                h_t = xp.tile([128, DIM], BF16, tag="h_t")
                _ln_apply(nc, sp, x_t, h_t, eps_col)
                for c in range(6):
                    ps_t = psT.tile([128, 128], BF16, tag="tp")
                    nc.tensor.transpose(ps_t, h_t[:, c * 128:(c + 1) * 128], ident)
                    nc.vector.tensor_copy(hT[:, c, t * 128:(t + 1) * 128], ps_t)

            # ---- phase B+C: per image QKV + attention ---------------------
            nc.vector.memset(attn_oT[:, :, NTOK:NTOKP], 0.0)
            for i in range(NB):
                qcols = slice(i * N, (i + 1) * N)
                qT_i = ip.tile([128, HEADS, N], BF16, tag="qT")
                kT_i = ip.tile([128, HEADS, N], BF16, tag="kT")
                v_i = ip.tile([128, 2, HEADS, 128], BF16, tag="v")
                qe = qext_d[:]
                nc.sync.dma_start(
                    qT_i[64:64 + KEXT, :, :],
                    bass.AP(tensor=qe.tensor, offset=qe.offset,
                            ap=[list(qe.ap[0]), [0, HEADS], list(qe.ap[1])]))
                nc.sync.dma_start(kT_i[64:64 + KEXT, :, :],
                                  kext_d[:].rearrange("h j t -> j h t"))
                nc.vector.memset(v_i[:, :, :, 64:128], 1.0)
                # q/k feature-major into head groups (rows 0:64)
                for m in range(12):
                    ps = psA.tile([128, 512], F32, tag="mm512")
                    for c in range(6):
                        nc.tensor.matmul(ps[:, 0:N],
                                         wqkv_sb[:, c, m * 128:(m + 1) * 128],
                                         hT[:, c, qcols],
                                         start=(c == 0), stop=(c == 5))
                    dst = qT_i if m < 6 else kT_i
                    hh = 2 * (m % 6)
                    nc.vector.tensor_scalar_add(
                        out=dst[0:64, hh, :], in0=ps[0:64, 0:N],
                        scalar1=qkb_sb[0:64, m:m + 1])
                    nc.vector.tensor_scalar_add(
                        out=dst[0:64, hh + 1, :], in0=ps[64:128, 0:N],
                        scalar1=qkb_sb[64:128, m:m + 1])
                # v token-major (per 128-token subtile of this image)
                for st in range(2):
                    tok0 = i * N + st * 128
                    ksz = min(128, (i + 1) * N - tok0)
                    for ns, w in enumerate([512, 256]):
                        ps = psA.tile([128, 512], F32, tag="mm512")
                        for c in range(6):
                            nc.tensor.matmul(
                                ps[0:ksz, 0:w],
                                hT[:, c, tok0:tok0 + ksz],
                                wqkv_sb[:, c, 2 * DIM + ns * 512:
                                        2 * DIM + ns * 512 + w],
                                start=(c == 0), stop=(c == 5))
                        nh = w // 64
                        nc.vector.tensor_copy(
                            v_i[0:ksz, st, ns * 8:ns * 8 + nh, 0:64],
                            ps[0:ksz, 0:w].rearrange(
                                "k (h d) -> k h d", d=64))
                # attention
                for h in range(HEADS):
                    pts = []
                    for st in range(2):
                        tok0 = i * N + st * 128
                        ksz = min(128, (i + 1) * N - tok0)
                        lt = tok0 - i * N
                        ps_s = psS.tile([128, N], F32, tag="att")
                        nc.tensor.matmul(ps_s[0:ksz],
                                         kT_i[0:64 + KEXT, h, lt:lt + ksz],
                                         qT_i[0:64 + KEXT, h, :],
                                         start=True, stop=True)
                        p_t = pp.tile([128, N], BF16, tag="p_t")
                        nc.scalar.activation(p_t[0:ksz], ps_s[0:ksz], Act.Exp)
                        pts.append((p_t, ksz))
                    ps_av = psS.tile([128, N], F32, tag="att")
                    for st, (p_t, ksz) in enumerate(pts):
                        nc.tensor.matmul(ps_av,
                                         v_i[0:ksz, st, h, :],
                                         p_t[0:ksz],
                                         start=(st == 0), stop=(st == 1))
                    # normalize: rows 0:64 / rows 64:128 -> attn_oT head slot
                    den = pp.tile([128, N], F32, tag="den")
                    nc.vector.reciprocal(den[0:64], ps_av[64:128])
                    nc.vector.tensor_tensor(
                        out=attn_oT[(h % 2) * 64:(h % 2) * 64 + 64, h // 2, qcols],
                        in0=ps_av[0:64], in1=den[0:64],
                        op=Alu.mult)

            # ---- phase D: proj + residual -> x1 (DRAM) --------------------
            for t in range(NTILES):
                rows = min(128, NTOK - t * 128)
                x_t = xp.tile([128, DIM], F32, tag="x_t")
                if rows < 128:
                    nc.vector.memset(x_t, 0.0)
                nc.sync.dma_start(x_t[0:rows], x_d[t * 128:t * 128 + rows])
                nc.vector.tensor_tensor(out=x_t, in0=x_t, in1=pb_sb, op=Alu.add)
                x1_t = xp.tile([128, DIM], F32, tag="y_sb")
                for ns, w in enumerate([512, 256]):
                    sl = slice(ns * 512, ns * 512 + w)
                    ps = psA.tile([128, 512], F32, tag="mm512")
                    for c in range(6):
                        nc.tensor.matmul(ps[:, 0:w],
                                         attn_oT[:, c, t * 128:(t + 1) * 128],
                                         wproj_sb[:, c, sl],
                                         start=(c == 0), stop=(c == 5))
                    nc.vector.tensor_tensor(out=x1_t[:, sl], in0=ps[:, 0:w],
                                            in1=x_t[:, sl], op=Alu.add)
                nc.sync.dma_start(x1_dram[t * 128:(t + 1) * 128], x1_t)

            # ---- phase E: LN2 + transpose to h2T --------------------------
            for t in range(NTILES):
                x_t = xp.tile([128, DIM], F32, tag="x_t")
                nc.sync.dma_start(x_t, x1_dram[t * 128:(t + 1) * 128])
                h_t = xp.tile([128, DIM], BF16, tag="h_t")
                _ln_apply(nc, sp, x_t, h_t, eps_col)
                for c in range(6):
                    ps_t = psT.tile([128, 128], BF16, tag="tp")
                    nc.tensor.transpose(ps_t, h_t[:, c * 128:(c + 1) * 128], ident)
                    nc.vector.tensor_copy(h2T[:, c, t * 128:(t + 1) * 128], ps_t)

            # ---- phase F: MLP ---------------------------------------------
            w1_sb = wp.tile([128, 6, HID], BF16, tag="wbig")
            nc.sync.dma_start(w1_sb, w1_d[:])
            w2_sb = wp.tile([128, 24, DIM], BF16, tag="wbig")
            nc.sync.dma_start(w2_sb, w2_d[:])
            for nsi, w in enumerate(NSL2):
                col0 = nsi * 256
                gT = gp.tile([128, 24, 256], BF16, tag="gT")
                for mc in range(24):
                    ps = psA.tile([128, 512], F32, tag="mm512")
                    for c in range(6):
                        nc.tensor.matmul(ps[:, 0:w],
                                         w1_sb[:, c, mc * 128:(mc + 1) * 128],
                                         h2T[:, c, col0:col0 + w],
                                         start=(c == 0), stop=(c == 5))
                    nc.scalar.activation(gT[:, mc, 0:w], ps[:, 0:w], Act.Gelu,
                                         bias=b1_sb[:, mc:mc + 1])
                for tt in range(w // 128):
                    t = nsi * 2 + tt
                    rows = min(128, NTOK - t * 128)
                    x_t = xp.tile([128, DIM], F32, tag="x_t")
                    nc.sync.dma_start(x_t, x1_dram[t * 128:(t + 1) * 128])
                    y_sb = xp.tile([128, DIM], F32, tag="y_sb")
                    for ns2, w2 in enumerate([512, 256]):
                        sl = slice(ns2 * 512, ns2 * 512 + w2)
                        ps = psA.tile([128, 512], F32, tag="mm512")
                        for kc in range(24):
                            nc.tensor.matmul(ps[:, 0:w2],
                                             gT[:, kc, tt * 128:(tt + 1) * 128],
                                             w2_sb[:, kc, sl],
                                             start=(kc == 0), stop=(kc == 23))
                        nc.vector.tensor_tensor(out=y_sb[:, sl], in0=ps[:, 0:w2],
                                                in1=x_t[:, sl], op=Alu.add)
                    if has_f2b:
                        nc.vector.tensor_tensor(out=y_sb, in0=y_sb, in1=f2b_sb,
                                                op=Alu.add)
                    nc.sync.dma_start(y_d[t * 128:t * 128 + rows], y_sb[0:rows])

    nc.compile()
    return nc


def kernel(**inputs) -> np.ndarray:
    x = np.asarray(inputs["x"], np.float32)          # [64, 197, 768]
    consts = _host_prep(inputs)
    key = ("blk", consts["has_f2b"])
    if key not in _nc_cache:
        _nc_cache[key] = _build(consts["has_f2b"])
    nc = _nc_cache[key]

    shared = {k: consts[k] for k in ("wqkv", "qkb", "wproj", "pb", "w1",
                                     "b1c", "w2", "f2b", "qext", "kext")}
    if not consts["has_f2b"]:
        shared.pop("f2b")
    in_maps = []
    for c in range(NCORES):
        m = dict(shared)
        m["x"] = np.ascontiguousarray(
            x[c * NB:(c + 1) * NB].reshape(NTOK, DIM))
        in_maps.append(m)

    res = run_bass_kernel_spmd(nc, in_maps, core_ids=list(range(NCORES)))
    out = np.empty((B, N, DIM), np.float32)
    for c in range(NCORES):
        out[c * NB:(c + 1) * NB] = res.results[c]["y"].reshape(NB, N, DIM)
    return out
